# revision 8
# baseline (speedup 1.0000x reference)
"""Batched normalized-gram kernel for 8 TRN2 NeuronCores.

reference:  x (64, 2, 512, 512) fp32
    x0 = x[:, 0]                               (B=64, V=512, F=512)
    n  = sqrt(sum(x0^2, axis=(0, 2)))          (V,)
    out[b] = (x0[b] @ x0[b].T) / outer(n, n)   (B, V, V)

gram[b,i,j]/(n_i n_j) == (x0[b,i,:]/n_i) . (x0[b,j,:]/n_j), so the host
prescales rows by 1/n once and the device work is a pure batched symmetric
matmul out[b] = y[b] @ y[b].T.

Device-side structure (per core, 8 batches):
  * operands shipped as fp16 — halves input DMA, full-rate PE, fp32 PSUM.
  * output is symmetric: device computes only the upper block-triangle
    (row-block mi covers columns mi*128..511), host mirrors the rest.
  * ONE input DMA per batch (batch 0: two half DMAs so compute starts as
    soon as the first half lands): host pre-interleaves y[b].T into a
    [128, 2048] layout (z[b, p, ki*512+v] = yT[b, ki*128+p, v]) so each
    batch streams as 128 x 4KiB contiguous descriptors.  Keeps the SP
    sequencer (~600ns config per DMA) far ahead of the PE.
  * input prefetch 5 batches deep — absorbs the ~3us DMA delivery latency
    (config+DGE+transfer+sem) without ever stalling the PE.
  * packed-triangle output staged in SBUF as fp16 (halves output DMA),
    one DMA per batch (last batch: two, so the final transfer is small),
    unpacked/mirrored on host.
  * PSUM->SBUF cast-copies split DVE (mi=0,2) / ACT (mi=1,3).
  * dummy warm-up matmuls on a zeroed tile run while batch 0's input DMA
    is in flight: PE is busy from the first user instruction, HAM
    un-throttles to 2.4 GHz by the time real matmuls begin.

Sharding: data-parallel over batch — 8 batches per core, no collectives.
"""

import numpy as np

B, T, V, F = 64, 2, 512, 512
NCORES = 8
BPC = B // NCORES  # batches per core
NBLK = V // 128  # 4 row-blocks
N_WARM = 6  # warm-up matmuls (N=512 each) before real work

# packed upper-triangle segment offsets: row-block mi holds cols mi*128..511
SEG_OFF = [0]
for _mi in range(NBLK):
    SEG_OFF.append(SEG_OFF[-1] + V - 128 * _mi)
SEG_TOTAL = SEG_OFF[-1]  # 1280

_NC = None


def _build_nc():
    import concourse.mybir as mybir
    import concourse.tile as tile
    from concourse import bacc

    f32 = mybir.dt.float32
    f16 = mybir.dt.float16

    nc = bacc.Bacc(target_bir_lowering=False)
    z = nc.declare_dram_parameter("z", [BPC, 128, NBLK * V], f16, isOutput=False)
    outp = nc.declare_dram_parameter(
        "outp", [BPC, 128, SEG_TOTAL], f16, isOutput=True
    )

    def copy_seg(ot, mi, ps):
        seg = ot[:, SEG_OFF[mi] : SEG_OFF[mi] + (V - 128 * mi)]
        if mi % 2 == 0:
            nc.vector.tensor_copy(out=seg, in_=ps)
        else:
            nc.scalar.copy(out=seg, in_=ps)

    with tile.TileContext(nc) as tc:
        with (
            tc.tile_pool(name="boot", bufs=1) as boot_pool,
            tc.tile_pool(name="inp", bufs=5) as inp_pool,
            tc.tile_pool(name="psum", bufs=8, space="PSUM") as psum_pool,
            tc.tile_pool(name="outp", bufs=3) as out_pool,
        ):
            # PE warm-up while batch 0's input DMA is in flight
            wt = boot_pool.tile([128, V], f16, tag="warm")
            nc.gpsimd.memset(wt, 0.0)
            wps = psum_pool.tile([128, V], f32, tag="ps")
            for _ in range(N_WARM):
                nc.tensor.matmul(wps, lhsT=wt[:, 0:128], rhs=wt, start=True, stop=True)

            # batches 0-2: four quarter-tiles each, ki-outer matmuls — the
            # PE only ever waits on one 128KiB quarter, not a whole batch,
            # so the input ring can build its pipeline lead without stalls
            N_FINE = 3
            for b in range(N_FINE):
                zh = []
                for h in range(NBLK):
                    t = boot_pool.tile([128, V], f16, tag=f"z{b}_{h}", name=f"z{b}_{h}")
                    nc.sync.dma_start(out=t, in_=z[b, :, h * V : (h + 1) * V])
                    zh.append(t)
                psb = [
                    psum_pool.tile(
                        [128, V - 128 * mi], f32, tag="ps", name=f"ps{b}_{mi}"
                    )
                    for mi in range(NBLK)
                ]
                ot = out_pool.tile([128, SEG_TOTAL], f16, tag="ot", name=f"ot{b}")
                for ki in range(NBLK):
                    src = zh[ki]
                    for mi in range(NBLK):
                        nc.tensor.matmul(
                            psb[mi],
                            lhsT=src[:, mi * 128 : (mi + 1) * 128],
                            rhs=src[:, mi * 128 :],
                            start=(ki == 0),
                            stop=(ki == NBLK - 1),
                        )
                for mi in range(NBLK):
                    copy_seg(ot, mi, psb[mi])
                nc.scalar.dma_start(out=outp[b], in_=ot)

            for b in range(N_FINE, BPC):
                last = b == BPC - 1
                zt = inp_pool.tile([128, NBLK * V], f16, tag="z")
                nc.sync.dma_start(out=zt, in_=z[b])
                ot = out_pool.tile([128, SEG_TOTAL], f16, tag="ot")
                for mi in range(NBLK):
                    n_cols = V - 128 * mi
                    ps = psum_pool.tile([128, n_cols], f32, tag="ps")
                    for ki in range(NBLK):
                        base = ki * V + mi * 128
                        nc.tensor.matmul(
                            ps,
                            lhsT=zt[:, base : base + 128],
                            rhs=zt[:, base : (ki + 1) * V],
                            start=(ki == 0),
                            stop=(ki == NBLK - 1),
                        )
                    if last:
                        # last batch: every copy on DVE, one output DMA per
                        # segment so the post-loop tail is a single tiny
                        # transfer on the otherwise-idle Sync ring
                        seg = ot[:, SEG_OFF[mi] : SEG_OFF[mi] + n_cols]
                        nc.vector.tensor_copy(out=seg, in_=ps)
                        if mi < NBLK - 1:
                            nc.scalar.dma_start(
                                out=outp[b, :, SEG_OFF[mi] : SEG_OFF[mi + 1]],
                                in_=seg,
                            )
                        else:
                            nc.sync.dma_start(
                                out=outp[b, :, SEG_OFF[mi] : SEG_OFF[mi + 1]],
                                in_=seg,
                            )
                    else:
                        copy_seg(ot, mi, ps)
                if not last:
                    nc.scalar.dma_start(out=outp[b], in_=ot)
    if not nc.is_finalized():
        nc.finalize()
    return nc


def _get_nc():
    global _NC
    if _NC is None:
        _NC = _build_nc()
    return _NC


def _prep_shards(x: np.ndarray) -> np.ndarray:
    x = np.ascontiguousarray(np.asarray(x, dtype=np.float32))
    x0 = x[:, 0]  # (B, V, F)
    ss = np.einsum("bvf,bvf->v", x0, x0, optimize=True)
    inv_n = (1.0 / np.sqrt(ss)).astype(np.float32)
    y = x0 * inv_n[None, :, None]
    # z[b, p, ki*512 + v] = y[b, v, ki*128 + p]: each batch is one
    # [128 partitions x 4096B-contiguous] DMA on device
    z = y.reshape(B, V, NBLK, 128).transpose(0, 3, 2, 1).reshape(B, 128, NBLK * V)
    return np.ascontiguousarray(z.astype(np.float16))


def kernel(x: np.ndarray, _trace: bool = False, _trace_out: list | None = None):
    from concourse.bass_utils import run_bass_kernel_spmd

    z = _prep_shards(x)
    nc = _get_nc()
    in_maps = [{"z": z[c * BPC : (c + 1) * BPC]} for c in range(NCORES)]
    res = run_bass_kernel_spmd(
        nc, in_maps, core_ids=list(range(NCORES)), trace=_trace
    )
    if _trace_out is not None:
        _trace_out.append(res)
    packed = np.concatenate(
        [np.asarray(res.results[c]["outp"]) for c in range(NCORES)], axis=0
    )  # (B, 128, 1280) fp16
    full = np.empty((B, V, V), dtype=np.float32)
    for mi in range(NBLK):
        full[:, mi * 128 : (mi + 1) * 128, mi * 128 :] = packed[
            :, :, SEG_OFF[mi] : SEG_OFF[mi + 1]
        ].astype(np.float32)
    # mirror the upper block-triangle down
    for mi in range(NBLK):
        for nj in range(mi + 1, NBLK):
            full[:, nj * 128 : (nj + 1) * 128, mi * 128 : (mi + 1) * 128] = (
                np.swapaxes(
                    full[:, mi * 128 : (mi + 1) * 128, nj * 128 : (nj + 1) * 128],
                    1,
                    2,
                )
            )
    return full


# revision 11
# speedup vs baseline: 1.0399x; 1.0399x over previous
"""Batched normalized-gram kernel for 8 TRN2 NeuronCores.

reference:  x (64, 2, 512, 512) fp32
    x0 = x[:, 0]                               (B=64, V=512, F=512)
    n  = sqrt(sum(x0^2, axis=(0, 2)))          (V,)
    out[b] = (x0[b] @ x0[b].T) / outer(n, n)   (B, V, V)

gram[b,i,j]/(n_i n_j) == (x0[b,i,:]/n_i) . (x0[b,j,:]/n_j), so the host
prescales rows by 1/n once and the device work is a pure batched symmetric
matmul out[b] = y[b] @ y[b].T.

Device-side structure (per core, 8 batches):
  * operands shipped as fp16 — halves input DMA, full-rate PE, fp32 PSUM.
  * output is symmetric: device computes only the upper block-triangle
    (row-block mi covers columns mi*128..511), host mirrors the rest.
  * ONE input DMA per batch (batch 0: two half DMAs so compute starts as
    soon as the first half lands): host pre-interleaves y[b].T into a
    [128, 2048] layout (z[b, p, ki*512+v] = yT[b, ki*128+p, v]) so each
    batch streams as 128 x 4KiB contiguous descriptors.  Keeps the SP
    sequencer (~600ns config per DMA) far ahead of the PE.
  * input prefetch 5 batches deep — absorbs the ~3us DMA delivery latency
    (config+DGE+transfer+sem) without ever stalling the PE.
  * packed-triangle output staged in SBUF as fp16 (halves output DMA),
    one DMA per batch (last batch: two, so the final transfer is small),
    unpacked/mirrored on host.
  * PSUM->SBUF cast-copies split DVE (mi=0,2) / ACT (mi=1,3).
  * dummy warm-up matmuls on a zeroed tile run while batch 0's input DMA
    is in flight: PE is busy from the first user instruction, HAM
    un-throttles to 2.4 GHz by the time real matmuls begin.

Sharding: data-parallel over batch — 8 batches per core, no collectives.
"""

import numpy as np

B, T, V, F = 64, 2, 512, 512
NCORES = 8
BPC = B // NCORES  # batches per core
NBLK = V // 128  # 4 row-blocks
N_WARM = 6  # warm-up matmuls (N=512 each) before real work

# packed upper-triangle segment offsets: row-block mi holds cols mi*128..511
SEG_OFF = [0]
for _mi in range(NBLK):
    SEG_OFF.append(SEG_OFF[-1] + V - 128 * _mi)
SEG_TOTAL = SEG_OFF[-1]  # 1280

_NC = None


def _build_nc():
    import concourse.mybir as mybir
    import concourse.tile as tile
    from concourse import bacc

    f32 = mybir.dt.float32
    f16 = mybir.dt.float16

    nc = bacc.Bacc(target_bir_lowering=False)
    z = nc.declare_dram_parameter("z", [BPC, 128, NBLK * V], f16, isOutput=False)
    outp = nc.declare_dram_parameter(
        "outp", [BPC, 128, SEG_TOTAL], f16, isOutput=True
    )

    def copy_seg(ot, mi, ps):
        seg = ot[:, SEG_OFF[mi] : SEG_OFF[mi] + (V - 128 * mi)]
        if mi % 2 == 0:
            nc.vector.tensor_copy(out=seg, in_=ps)
        else:
            nc.scalar.copy(out=seg, in_=ps)

    with tile.TileContext(nc) as tc:
        with (
            tc.tile_pool(name="boot", bufs=1) as boot_pool,
            tc.tile_pool(name="inp", bufs=5) as inp_pool,
            tc.tile_pool(name="psum", bufs=8, space="PSUM") as psum_pool,
            tc.tile_pool(name="outp", bufs=3) as out_pool,
        ):
            # PE warm-up while batch 0's input DMA is in flight
            wt = boot_pool.tile([128, V], f16, tag="warm")
            nc.gpsimd.memset(wt, 0.0)
            wps = psum_pool.tile([128, V], f32, tag="ps")
            for _ in range(N_WARM):
                nc.tensor.matmul(wps, lhsT=wt[:, 0:128], rhs=wt, start=True, stop=True)

            # batches 0-2 load in fine pieces (batch 0: quarters, 1-2:
            # halves) with ki-outer matmuls — the PE only ever waits on one
            # piece, so the input ring builds its pipeline lead stall-free.
            # 8 fine configs keep the Sync sequencer well ahead of the PE
            # (v4 showed 12 configs starve batches 3-4).
            fine_pieces = {}  # b -> [(tile, ki_lo)]
            for b, npc in ((0, NBLK), (1, 2), (2, 2)):
                kper = NBLK // npc
                ps_ = []
                for h in range(npc):
                    t = boot_pool.tile(
                        [128, kper * V], f16, tag=f"z{b}_{h}", name=f"z{b}_{h}"
                    )
                    nc.sync.dma_start(
                        out=t, in_=z[b, :, h * kper * V : (h + 1) * kper * V]
                    )
                    ps_.append((t, h * kper))
                fine_pieces[b] = ps_

            for b in sorted(fine_pieces):
                pieces = fine_pieces[b]
                psb = [
                    psum_pool.tile(
                        [128, V - 128 * mi], f32, tag="ps", name=f"ps{b}_{mi}"
                    )
                    for mi in range(NBLK)
                ]
                ot = out_pool.tile([128, SEG_TOTAL], f16, tag="ot", name=f"ot{b}")
                for ki in range(NBLK):
                    src, ki_lo = next(
                        (t, lo) for t, lo in reversed(pieces) if lo <= ki
                    )
                    off = (ki - ki_lo) * V
                    for mi in range(NBLK):
                        nc.tensor.matmul(
                            psb[mi],
                            lhsT=src[:, off + mi * 128 : off + (mi + 1) * 128],
                            rhs=src[:, off + mi * 128 : off + V],
                            start=(ki == 0),
                            stop=(ki == NBLK - 1),
                        )
                for mi in range(NBLK):
                    copy_seg(ot, mi, psb[mi])
                nc.scalar.dma_start(out=outp[b], in_=ot)

            for b in range(len(fine_pieces), BPC):
                last = b == BPC - 1
                zt = inp_pool.tile([128, NBLK * V], f16, tag="z")
                nc.sync.dma_start(out=zt, in_=z[b])
                ot = out_pool.tile([128, SEG_TOTAL], f16, tag="ot")
                for mi in range(NBLK):
                    n_cols = V - 128 * mi
                    ps = psum_pool.tile([128, n_cols], f32, tag="ps")
                    for ki in range(NBLK):
                        base = ki * V + mi * 128
                        nc.tensor.matmul(
                            ps,
                            lhsT=zt[:, base : base + 128],
                            rhs=zt[:, base : (ki + 1) * V],
                            start=(ki == 0),
                            stop=(ki == NBLK - 1),
                        )
                    if last:
                        # last batch: every copy on DVE, one output DMA per
                        # segment so the post-loop tail is a single tiny
                        # transfer on the otherwise-idle Sync ring
                        seg = ot[:, SEG_OFF[mi] : SEG_OFF[mi] + n_cols]
                        nc.vector.tensor_copy(out=seg, in_=ps)
                        if mi < NBLK - 1:
                            nc.scalar.dma_start(
                                out=outp[b, :, SEG_OFF[mi] : SEG_OFF[mi + 1]],
                                in_=seg,
                            )
                        else:
                            nc.sync.dma_start(
                                out=outp[b, :, SEG_OFF[mi] : SEG_OFF[mi + 1]],
                                in_=seg,
                            )
                    else:
                        copy_seg(ot, mi, ps)
                if not last:
                    nc.scalar.dma_start(out=outp[b], in_=ot)
    if not nc.is_finalized():
        nc.finalize()
    return nc


def _get_nc():
    global _NC
    if _NC is None:
        _NC = _build_nc()
    return _NC


def _prep_shards(x: np.ndarray) -> np.ndarray:
    x = np.ascontiguousarray(np.asarray(x, dtype=np.float32))
    x0 = x[:, 0]  # (B, V, F)
    ss = np.einsum("bvf,bvf->v", x0, x0, optimize=True)
    inv_n = (1.0 / np.sqrt(ss)).astype(np.float32)
    y = x0 * inv_n[None, :, None]
    # z[b, p, ki*512 + v] = y[b, v, ki*128 + p]: each batch is one
    # [128 partitions x 4096B-contiguous] DMA on device
    z = y.reshape(B, V, NBLK, 128).transpose(0, 3, 2, 1).reshape(B, 128, NBLK * V)
    return np.ascontiguousarray(z.astype(np.float16))


def kernel(x: np.ndarray, _trace: bool = False, _trace_out: list | None = None):
    from concourse.bass_utils import run_bass_kernel_spmd

    z = _prep_shards(x)
    nc = _get_nc()
    in_maps = [{"z": z[c * BPC : (c + 1) * BPC]} for c in range(NCORES)]
    res = run_bass_kernel_spmd(
        nc, in_maps, core_ids=list(range(NCORES)), trace=_trace
    )
    if _trace_out is not None:
        _trace_out.append(res)
    packed = np.concatenate(
        [np.asarray(res.results[c]["outp"]) for c in range(NCORES)], axis=0
    )  # (B, 128, 1280) fp16
    full = np.empty((B, V, V), dtype=np.float32)
    for mi in range(NBLK):
        full[:, mi * 128 : (mi + 1) * 128, mi * 128 :] = packed[
            :, :, SEG_OFF[mi] : SEG_OFF[mi + 1]
        ].astype(np.float32)
    # mirror the upper block-triangle down
    for mi in range(NBLK):
        for nj in range(mi + 1, NBLK):
            full[:, nj * 128 : (nj + 1) * 128, mi * 128 : (mi + 1) * 128] = (
                np.swapaxes(
                    full[:, mi * 128 : (mi + 1) * 128, nj * 128 : (nj + 1) * 128],
                    1,
                    2,
                )
            )
    return full


# revision 15
# speedup vs baseline: 1.0901x; 1.0483x over previous
"""Batched normalized-gram kernel for 8 TRN2 NeuronCores.

reference:  x (64, 2, 512, 512) fp32
    x0 = x[:, 0]                               (B=64, V=512, F=512)
    n  = sqrt(sum(x0^2, axis=(0, 2)))          (V,)
    out[b] = (x0[b] @ x0[b].T) / outer(n, n)   (B, V, V)

gram[b,i,j]/(n_i n_j) == (x0[b,i,:]/n_i) . (x0[b,j,:]/n_j), so the host
prescales rows by 1/n once and the device work is a pure batched symmetric
matmul out[b] = y[b] @ y[b].T.

Device-side structure (per core, 8 batches):
  * operands shipped as fp16 — halves input DMA, full-rate PE, fp32 PSUM.
  * output is symmetric: device computes only the upper block-triangle
    (row-block mi covers columns mi*128..511), host mirrors the rest.
  * ONE input DMA per batch (batch 0: two half DMAs so compute starts as
    soon as the first half lands): host pre-interleaves y[b].T into a
    [128, 2048] layout (z[b, p, ki*512+v] = yT[b, ki*128+p, v]) so each
    batch streams as 128 x 4KiB contiguous descriptors.  Keeps the SP
    sequencer (~600ns config per DMA) far ahead of the PE.
  * input prefetch 5 batches deep — absorbs the ~3us DMA delivery latency
    (config+DGE+transfer+sem) without ever stalling the PE.
  * packed-triangle output staged in SBUF as fp16 (halves output DMA),
    one DMA per batch (last batch: two, so the final transfer is small),
    unpacked/mirrored on host.
  * PSUM->SBUF cast-copies split DVE (mi=0,2) / ACT (mi=1,3).
  * dummy warm-up matmuls on a zeroed tile run while batch 0's input DMA
    is in flight: PE is busy from the first user instruction, HAM
    un-throttles to 2.4 GHz by the time real matmuls begin.

Sharding: data-parallel over batch — 8 batches per core, no collectives.
"""

import numpy as np

B, T, V, F = 64, 2, 512, 512
NCORES = 8
BPC = B // NCORES  # batches per core
NBLK = V // 128  # 4 row-blocks
N_WARM = 6  # warm-up matmuls (N=512 each) before real work

# packed upper-triangle segment offsets: row-block mi holds cols mi*128..511
SEG_OFF = [0]
for _mi in range(NBLK):
    SEG_OFF.append(SEG_OFF[-1] + V - 128 * _mi)
SEG_TOTAL = SEG_OFF[-1]  # 1280

_NC = None


def _build_nc():
    import concourse.mybir as mybir
    import concourse.tile as tile
    from concourse import bacc

    f32 = mybir.dt.float32
    f16 = mybir.dt.float16

    nc = bacc.Bacc(target_bir_lowering=False)
    z = nc.declare_dram_parameter("z", [BPC, 128, NBLK * V], f16, isOutput=False)
    outp = nc.declare_dram_parameter(
        "outp", [BPC, 128, SEG_TOTAL], f16, isOutput=True
    )

    def copy_seg(ot, mi, ps, b):
        # one copy engine per batch: a single writer per ot tile avoids the
        # cross-engine write-ordering chain the framework otherwise inserts
        seg = ot[:, SEG_OFF[mi] : SEG_OFF[mi] + (V - 128 * mi)]
        if b % 2 == 1:
            nc.vector.tensor_copy(out=seg, in_=ps)
        else:
            nc.scalar.copy(out=seg, in_=ps)

    with tile.TileContext(nc) as tc:
        with (
            tc.tile_pool(name="boot", bufs=1) as boot_pool,
            tc.tile_pool(name="inp", bufs=5) as inp_pool,
            tc.tile_pool(name="psum", bufs=8, space="PSUM") as psum_pool,
            tc.tile_pool(name="outp", bufs=3) as out_pool,
        ):
            # PE warm-up while batch 0's input DMA is in flight: a few big
            # matmuls, then short ones so the handoff to real work has at
            # most ~100ns of PE idle (a longer gap can reset the HAM busy
            # window and leave the clock throttled for the whole ramp)
            wt = boot_pool.tile([128, V], f16, tag="warm")
            nc.gpsimd.memset(wt, 0.0)
            wps = psum_pool.tile([128, V], f32, tag="ps")
            for _ in range(5):
                nc.tensor.matmul(wps, lhsT=wt[:, 0:128], rhs=wt, start=True, stop=True)
            for _ in range(10):
                nc.tensor.matmul(
                    wps[:, 0:128], lhsT=wt[:, 0:128], rhs=wt[:, 0:128],
                    start=True, stop=True,
                )

            # batches 0-2 load in fine pieces (batch 0: quarters, 1-2:
            # halves) with ki-outer matmuls — the PE only ever waits on one
            # piece, so the input ring builds its pipeline lead stall-free.
            # 8 fine configs keep the Sync sequencer well ahead of the PE
            # (v4 showed 12 configs starve batches 3-4).
            fine_pieces = {}  # b -> [(tile, ki_lo)]
            for b, npc in ((0, NBLK), (1, 2), (2, 2)):
                kper = NBLK // npc
                ps_ = []
                for h in range(npc):
                    t = boot_pool.tile(
                        [128, kper * V], f16, tag=f"z{b}_{h}", name=f"z{b}_{h}"
                    )
                    nc.sync.dma_start(
                        out=t, in_=z[b, :, h * kper * V : (h + 1) * kper * V]
                    )
                    ps_.append((t, h * kper))
                fine_pieces[b] = ps_

            for b in sorted(fine_pieces):
                pieces = fine_pieces[b]
                psb = [
                    psum_pool.tile(
                        [128, V - 128 * mi], f32, tag="ps", name=f"ps{b}_{mi}"
                    )
                    for mi in range(NBLK)
                ]
                ot = out_pool.tile([128, SEG_TOTAL], f16, tag="ot", name=f"ot{b}")
                for ki in range(NBLK):
                    src, ki_lo = next(
                        (t, lo) for t, lo in reversed(pieces) if lo <= ki
                    )
                    off = (ki - ki_lo) * V
                    for mi in range(NBLK):
                        nc.tensor.matmul(
                            psb[mi],
                            lhsT=src[:, off + mi * 128 : off + (mi + 1) * 128],
                            rhs=src[:, off + mi * 128 : off + V],
                            start=(ki == 0),
                            stop=(ki == NBLK - 1),
                        )
                for mi in range(NBLK):
                    copy_seg(ot, mi, psb[mi], b)
                nc.scalar.dma_start(out=outp[b], in_=ot)

            for b in range(len(fine_pieces), BPC):
                last = b == BPC - 1
                zt = inp_pool.tile([128, NBLK * V], f16, tag="z")
                nc.sync.dma_start(out=zt, in_=z[b])
                ot = out_pool.tile([128, SEG_TOTAL], f16, tag="ot")
                for mi in range(NBLK):
                    n_cols = V - 128 * mi
                    ps = psum_pool.tile([128, n_cols], f32, tag="ps")
                    for ki in range(NBLK):
                        base = ki * V + mi * 128
                        nc.tensor.matmul(
                            ps,
                            lhsT=zt[:, base : base + 128],
                            rhs=zt[:, base : (ki + 1) * V],
                            start=(ki == 0),
                            stop=(ki == NBLK - 1),
                        )
                    if last:
                        # last batch: every copy on DVE, one output DMA per
                        # segment so the post-loop tail is a single tiny
                        # transfer on the otherwise-idle Sync ring
                        seg = ot[:, SEG_OFF[mi] : SEG_OFF[mi] + n_cols]
                        nc.vector.tensor_copy(out=seg, in_=ps)
                        if mi < NBLK - 1:
                            nc.scalar.dma_start(
                                out=outp[b, :, SEG_OFF[mi] : SEG_OFF[mi + 1]],
                                in_=seg,
                            )
                        else:
                            nc.sync.dma_start(
                                out=outp[b, :, SEG_OFF[mi] : SEG_OFF[mi + 1]],
                                in_=seg,
                            )
                    else:
                        copy_seg(ot, mi, ps, b)
                if not last:
                    nc.scalar.dma_start(out=outp[b], in_=ot)
    if not nc.is_finalized():
        nc.finalize()
    return nc


def _get_nc():
    global _NC
    if _NC is None:
        _NC = _build_nc()
    return _NC


def _prep_shards(x: np.ndarray) -> np.ndarray:
    x = np.ascontiguousarray(np.asarray(x, dtype=np.float32))
    x0 = x[:, 0]  # (B, V, F)
    ss = np.einsum("bvf,bvf->v", x0, x0, optimize=True)
    inv_n = (1.0 / np.sqrt(ss)).astype(np.float32)
    y = x0 * inv_n[None, :, None]
    # z[b, p, ki*512 + v] = y[b, v, ki*128 + p]: each batch is one
    # [128 partitions x 4096B-contiguous] DMA on device
    z = y.reshape(B, V, NBLK, 128).transpose(0, 3, 2, 1).reshape(B, 128, NBLK * V)
    return np.ascontiguousarray(z.astype(np.float16))


def kernel(x: np.ndarray, _trace: bool = False, _trace_out: list | None = None):
    from concourse.bass_utils import run_bass_kernel_spmd

    z = _prep_shards(x)
    nc = _get_nc()
    in_maps = [{"z": z[c * BPC : (c + 1) * BPC]} for c in range(NCORES)]
    res = run_bass_kernel_spmd(
        nc, in_maps, core_ids=list(range(NCORES)), trace=_trace
    )
    if _trace_out is not None:
        _trace_out.append(res)
    packed = np.concatenate(
        [np.asarray(res.results[c]["outp"]) for c in range(NCORES)], axis=0
    )  # (B, 128, 1280) fp16
    full = np.empty((B, V, V), dtype=np.float32)
    for mi in range(NBLK):
        full[:, mi * 128 : (mi + 1) * 128, mi * 128 :] = packed[
            :, :, SEG_OFF[mi] : SEG_OFF[mi + 1]
        ].astype(np.float32)
    # mirror the upper block-triangle down
    for mi in range(NBLK):
        for nj in range(mi + 1, NBLK):
            full[:, nj * 128 : (nj + 1) * 128, mi * 128 : (mi + 1) * 128] = (
                np.swapaxes(
                    full[:, mi * 128 : (mi + 1) * 128, nj * 128 : (nj + 1) * 128],
                    1,
                    2,
                )
            )
    return full


# revision 18
# speedup vs baseline: 1.1680x; 1.0715x over previous
"""Batched normalized-gram kernel for 8 TRN2 NeuronCores.

reference:  x (64, 2, 512, 512) fp32
    x0 = x[:, 0]                               (B=64, V=512, F=512)
    n  = sqrt(sum(x0^2, axis=(0, 2)))          (V,)
    out[b] = (x0[b] @ x0[b].T) / outer(n, n)   (B, V, V)

gram[b,i,j]/(n_i n_j) == (x0[b,i,:]/n_i) . (x0[b,j,:]/n_j), so the host
prescales rows by 1/n once and the device work is a pure batched symmetric
matmul out[b] = y[b] @ y[b].T.

Device-side structure (per core, 8 batches):
  * operands shipped as fp16 — halves input DMA, full-rate PE, fp32 PSUM.
  * output is symmetric: device computes only the upper block-triangle
    (row-block mi covers columns mi*128..511), host mirrors the rest.
  * ONE input DMA per batch (batch 0: two half DMAs so compute starts as
    soon as the first half lands): host pre-interleaves y[b].T into a
    [128, 2048] layout (z[b, p, ki*512+v] = yT[b, ki*128+p, v]) so each
    batch streams as 128 x 4KiB contiguous descriptors.  Keeps the SP
    sequencer (~600ns config per DMA) far ahead of the PE.
  * input prefetch 5 batches deep — absorbs the ~3us DMA delivery latency
    (config+DGE+transfer+sem) without ever stalling the PE.
  * packed-triangle output staged in SBUF as fp16 (halves output DMA),
    one DMA per batch (last batch: two, so the final transfer is small),
    unpacked/mirrored on host.
  * PSUM->SBUF cast-copies split DVE (mi=0,2) / ACT (mi=1,3).
  * dummy warm-up matmuls on a zeroed tile run while batch 0's input DMA
    is in flight: PE is busy from the first user instruction, HAM
    un-throttles to 2.4 GHz by the time real matmuls begin.

Sharding: data-parallel over batch — 8 batches per core, no collectives.
"""

import numpy as np

B, T, V, F = 64, 2, 512, 512
NCORES = 8
BPC = B // NCORES  # batches per core
NBLK = V // 128  # 4 row-blocks
N_WARM = 6  # warm-up matmuls (N=512 each) before real work

# packed upper-triangle segment offsets: row-block mi holds cols mi*128..511
SEG_OFF = [0]
for _mi in range(NBLK):
    SEG_OFF.append(SEG_OFF[-1] + V - 128 * _mi)
SEG_TOTAL = SEG_OFF[-1]  # 1280

_NC = None


def _build_nc():
    import concourse.mybir as mybir
    import concourse.tile as tile
    from concourse import bacc

    f32 = mybir.dt.float32
    f16 = mybir.dt.float16

    nc = bacc.Bacc(target_bir_lowering=False)
    z = nc.declare_dram_parameter("z", [BPC, 128, NBLK * V], f16, isOutput=False)
    outp = nc.declare_dram_parameter(
        "outp", [BPC, 128, SEG_TOTAL], f16, isOutput=True
    )

    def copy_seg(ot, mi, ps, b):
        # one copy engine per batch: a single writer per ot tile avoids the
        # cross-engine write-ordering chain the framework otherwise inserts
        seg = ot[:, SEG_OFF[mi] : SEG_OFF[mi] + (V - 128 * mi)]
        if b % 2 == 1:
            nc.vector.tensor_copy(out=seg, in_=ps)
        else:
            nc.scalar.copy(out=seg, in_=ps)

    with tile.TileContext(nc) as tc:
        with (
            tc.tile_pool(name="boot", bufs=1) as boot_pool,
            tc.tile_pool(name="inp", bufs=5) as inp_pool,
            tc.tile_pool(name="psum", bufs=8, space="PSUM") as psum_pool,
            tc.tile_pool(name="outp", bufs=8) as out_pool,
        ):
            # PE warm-up while batch 0's input DMA is in flight: a few big
            # matmuls, then short ones so the handoff to real work has at
            # most ~100ns of PE idle (a longer gap can reset the HAM busy
            # window and leave the clock throttled for the whole ramp)
            wt = boot_pool.tile([128, V], f16, tag="warm")
            nc.gpsimd.memset(wt, 0.0)
            wps = psum_pool.tile([128, V], f32, tag="ps")
            for _ in range(5):
                nc.tensor.matmul(wps, lhsT=wt[:, 0:128], rhs=wt, start=True, stop=True)
            for _ in range(10):
                nc.tensor.matmul(
                    wps[:, 0:128], lhsT=wt[:, 0:128], rhs=wt[:, 0:128],
                    start=True, stop=True,
                )

            # batches 0-2 load in fine pieces (batch 0: quarters, 1-2:
            # halves) with ki-outer matmuls — the PE only ever waits on one
            # piece, so the input ring builds its pipeline lead stall-free.
            # 8 fine configs keep the Sync sequencer well ahead of the PE
            # (v4 showed 12 configs starve batches 3-4).
            fine_pieces = {}  # b -> [(tile, ki_lo)]
            for b, npc in ((0, NBLK), (1, 2), (2, 2)):
                kper = NBLK // npc
                ps_ = []
                for h in range(npc):
                    t = boot_pool.tile(
                        [128, kper * V], f16, tag=f"z{b}_{h}", name=f"z{b}_{h}"
                    )
                    nc.sync.dma_start(
                        out=t, in_=z[b, :, h * kper * V : (h + 1) * kper * V]
                    )
                    ps_.append((t, h * kper))
                fine_pieces[b] = ps_

            # batches 3-7: whole-tile input DMAs, emitted BEFORE any output
            # DMA config lands on the Sync ring — HWDGE rings are strict
            # FIFO, so every input transfer completes before ring bandwidth
            # is spent on outputs (outputs have slack; inputs gate the PE)
            whole_zt = {}
            for b in range(len(fine_pieces), BPC):
                zt = inp_pool.tile([128, NBLK * V], f16, tag="z", name=f"zt{b}")
                nc.sync.dma_start(out=zt, in_=z[b])
                whole_zt[b] = zt

            for b in sorted(fine_pieces):
                pieces = fine_pieces[b]
                psb = [
                    psum_pool.tile(
                        [128, V - 128 * mi], f32, tag="ps", name=f"ps{b}_{mi}"
                    )
                    for mi in range(NBLK)
                ]
                ot = out_pool.tile([128, SEG_TOTAL], f16, tag="ot", name=f"ot{b}")
                for ki in range(NBLK):
                    src, ki_lo = next(
                        (t, lo) for t, lo in reversed(pieces) if lo <= ki
                    )
                    off = (ki - ki_lo) * V
                    for mi in range(NBLK):
                        nc.tensor.matmul(
                            psb[mi],
                            lhsT=src[:, off + mi * 128 : off + (mi + 1) * 128],
                            rhs=src[:, off + mi * 128 : off + V],
                            start=(ki == 0),
                            stop=(ki == NBLK - 1),
                        )
                for mi in range(NBLK):
                    copy_seg(ot, mi, psb[mi], b)
                # fine-batch outputs ride the Sync ring BEHIND all input
                # configs: they ship only once every input transfer is done
                nc.sync.dma_start(out=outp[b], in_=ot)

            for b in range(len(fine_pieces), BPC):
                last = b == BPC - 1
                zt = whole_zt[b]
                ot = out_pool.tile([128, SEG_TOTAL], f16, tag="ot")
                for mi in range(NBLK):
                    n_cols = V - 128 * mi
                    ps = psum_pool.tile([128, n_cols], f32, tag="ps")
                    for ki in range(NBLK):
                        base = ki * V + mi * 128
                        nc.tensor.matmul(
                            ps,
                            lhsT=zt[:, base : base + 128],
                            rhs=zt[:, base : (ki + 1) * V],
                            start=(ki == 0),
                            stop=(ki == NBLK - 1),
                        )
                    if last:
                        # last batch: every copy on DVE, one output DMA per
                        # segment so the post-loop tail is a single tiny
                        # transfer on the otherwise-idle Sync ring
                        seg = ot[:, SEG_OFF[mi] : SEG_OFF[mi] + n_cols]
                        nc.vector.tensor_copy(out=seg, in_=ps)
                        if mi < NBLK - 1:
                            nc.scalar.dma_start(
                                out=outp[b, :, SEG_OFF[mi] : SEG_OFF[mi + 1]],
                                in_=seg,
                            )
                        else:
                            nc.sync.dma_start(
                                out=outp[b, :, SEG_OFF[mi] : SEG_OFF[mi + 1]],
                                in_=seg,
                            )
                    else:
                        copy_seg(ot, mi, ps, b)
                if not last:
                    nc.scalar.dma_start(out=outp[b], in_=ot)
    if not nc.is_finalized():
        nc.finalize()
    return nc


def _get_nc():
    global _NC
    if _NC is None:
        _NC = _build_nc()
    return _NC


def _prep_shards(x: np.ndarray) -> np.ndarray:
    x = np.ascontiguousarray(np.asarray(x, dtype=np.float32))
    x0 = x[:, 0]  # (B, V, F)
    ss = np.einsum("bvf,bvf->v", x0, x0, optimize=True)
    inv_n = (1.0 / np.sqrt(ss)).astype(np.float32)
    y = x0 * inv_n[None, :, None]
    # z[b, p, ki*512 + v] = y[b, v, ki*128 + p]: each batch is one
    # [128 partitions x 4096B-contiguous] DMA on device
    z = y.reshape(B, V, NBLK, 128).transpose(0, 3, 2, 1).reshape(B, 128, NBLK * V)
    return np.ascontiguousarray(z.astype(np.float16))


def kernel(x: np.ndarray, _trace: bool = False, _trace_out: list | None = None):
    from concourse.bass_utils import run_bass_kernel_spmd

    z = _prep_shards(x)
    nc = _get_nc()
    in_maps = [{"z": z[c * BPC : (c + 1) * BPC]} for c in range(NCORES)]
    res = run_bass_kernel_spmd(
        nc, in_maps, core_ids=list(range(NCORES)), trace=_trace
    )
    if _trace_out is not None:
        _trace_out.append(res)
    packed = np.concatenate(
        [np.asarray(res.results[c]["outp"]) for c in range(NCORES)], axis=0
    )  # (B, 128, 1280) fp16
    full = np.empty((B, V, V), dtype=np.float32)
    for mi in range(NBLK):
        full[:, mi * 128 : (mi + 1) * 128, mi * 128 :] = packed[
            :, :, SEG_OFF[mi] : SEG_OFF[mi + 1]
        ].astype(np.float32)
    # mirror the upper block-triangle down
    for mi in range(NBLK):
        for nj in range(mi + 1, NBLK):
            full[:, nj * 128 : (nj + 1) * 128, mi * 128 : (mi + 1) * 128] = (
                np.swapaxes(
                    full[:, mi * 128 : (mi + 1) * 128, nj * 128 : (nj + 1) * 128],
                    1,
                    2,
                )
            )
    return full
